# revision 7
# baseline (speedup 1.0000x reference)
"""Trainium2 Bass kernel for nn_DCTLayer: 8x8 block DCT-II followed by its exact
inverse (torch_dct norm=None convention). The DCT->IDCT round trip is the
identity map in exact arithmetic, so the layer reduces to the block-layout
permutation (B, C, H, W) -> (B, C, 1, H, W) where out[b, c, 0] is the row-major
flatten of the (H/8, W/8, 8, 8) block view of the input. Computing the
permutation exactly is strictly more accurate than the reference's own fp32 FFT
round trip (rel err ~1e-7 against it).

Distribution (pure data parallelism over batch, 8 cores, no communication):
  - core k handles batches 4k..4k+4 = 12 images of 512x512 f32 (12 MiB).
  - Input viewed as [768, 4096]: each row chunk = 8 consecutive image rows
    (16 KiB, DRAM-contiguous) -> one SBUF partition.
  - On-chip shuffle per partition (vector engine, 4D access patterns):
    free-dim permutation (r, bw, c) -> (bw, r, c) with r=8, bw=64, c=8.
  - Output [768, 4096] is DRAM-contiguous per partition too, so both DMAs run
    at full descriptor efficiency. Binding resource: per-engine SDMA port rate
    (~26.5 GB/s x 16 engines) -> ~60 us conveyor per core.

Raw bass (no TileContext), minimal semaphore protocol:
  - all 6 tile loads issued up front, alternating the two HWDGE rings
    (SP=sync, ACT=scalar) so the DMA engines fill from both descriptor
    generators at once; the whole 96 KiB/partition input stays resident.
  - 2 half-tile shuffles per tile on vector, each into a dedicated out
    buffer column (96 KiB/partition out, no buffer reuse -> no waits).
  - stores split per half-tile across both rings behind the copies.
  - exactly one completion wait (gpsimd, single store sem) gates the NEFF
    epilogue; the wrapper re-clears all kernel sems at each execution's
    preamble, so no kernel-side sem cleanup is emitted at all.
"""

import numpy as np

_B, _C, _H, _W = 32, 3, 512, 512
_N_CORES = 8
_ROWS = (_B // _N_CORES) * _C * (_H // 8)  # 768 row chunks per core
_COLS = 8 * _W                             # 4096 f32 per chunk
_N_TILES = _ROWS // 128                    # 6 tiles of [128, 4096]

_nc_cache = None


def _build():
    import concourse.mybir as mybir
    from concourse import bacc

    nc = bacc.Bacc(
        "TRN2", target_bir_lowering=False, debug=False, num_devices=_N_CORES
    )
    x = nc.dram_tensor(
        "x", (_ROWS, _COLS), mybir.dt.float32, kind="ExternalInput"
    ).ap()
    y = nc.dram_tensor(
        "y", (_ROWS, _COLS), mybir.dt.float32, kind="ExternalOutput"
    ).ap()

    f32 = mybir.dt.float32
    with (
        nc.sbuf_tensor([128, _N_TILES * _COLS], f32) as tin,
        nc.sbuf_tensor([128, _N_TILES * _COLS], f32) as tout,
        nc.semaphore() as sem_ld_sp,   # loads on sync (SP ring)
        nc.semaphore() as sem_ld_act,  # loads on scalar (ACT ring)
        nc.semaphore() as sem_cp,      # vector copies
        nc.semaphore() as sem_st,      # stores, both rings
    ):
        # All 6 loads issued up front: L0,L2,L4 -> sync, L1,L3,L5 -> scalar.
        # tin column block t holds DRAM rows [128t, 128t+128).
        for t in range(_N_TILES):
            eng = nc.sync if t % 2 == 0 else nc.scalar
            sem = sem_ld_sp if t % 2 == 0 else sem_ld_act
            eng.dma_start(
                out=tin[:, t * _COLS:(t + 1) * _COLS],
                in_=x[t * 128:(t + 1) * 128, :],
                single_packet=True,
            ).then_inc(sem, 16)

        # Vector: per tile, 2 half-shuffles (bw split) into a dedicated out
        # buffer column (no reuse).
        for t in range(_N_TILES):
            sem = sem_ld_sp if t % 2 == 0 else sem_ld_act
            nc.vector.wait_ge(sem, 16 * (t // 2 + 1))
            src = tin[:, t * _COLS:(t + 1) * _COLS].rearrange(
                "p (r bw c) -> p bw r c", r=8, bw=64, c=8
            )
            dst = tout[:, t * _COLS:(t + 1) * _COLS].rearrange(
                "p (bw r c) -> p bw r c", bw=64, r=8, c=8
            )
            for s in range(2):
                bws = slice(s * 32, (s + 1) * 32)
                nc.vector.tensor_copy(out=dst[:, bws], in_=src[:, bws]).then_inc(
                    sem_cp, 1
                )

        # Stores: half-tile (t, s): s=0 -> scalar (ACT), s=1 -> sync (SP).
        # Store (t, s) needs copy count 2t + s + 1. Both rings inc sem_st.
        for t in range(_N_TILES):
            for s, eng in enumerate((nc.scalar, nc.sync)):
                eng.wait_ge(sem_cp, 2 * t + s + 1)
                eng.dma_start(
                    out=y[t * 128:(t + 1) * 128, s * 2048:(s + 1) * 2048],
                    in_=tout[:, t * _COLS + s * 2048:t * _COLS + (s + 1) * 2048],
                    single_packet=True,
                ).then_inc(sem_st, 16)

        # Single completion gate: all 12 stores (2 rings) counted on one sem.
        # The NEFF preamble re-clears the whole kernel sem range on every
        # execution, so no sem cleanup is needed here.
        nc.gpsimd.wait_ge(sem_st, 16 * 2 * _N_TILES)

        nc.compile()
    return nc


def kernel(x: np.ndarray) -> np.ndarray:
    from concourse import bass_utils

    global _nc_cache
    if _nc_cache is None:
        _nc_cache = _build()
    nc = _nc_cache

    x = np.ascontiguousarray(x, dtype=np.float32)
    assert x.shape == (_B, _C, _H, _W), x.shape
    xs = x.reshape(_N_CORES, _ROWS, _COLS)
    in_maps = [{"x": xs[k]} for k in range(_N_CORES)]
    res = bass_utils.run_bass_kernel_spmd(
        nc, in_maps, core_ids=list(range(_N_CORES))
    )
    ys = np.stack([res.results[k]["y"] for k in range(_N_CORES)], axis=0)
    return ys.reshape(_B, _C, 1, _H, _W)


# revision 8
# speedup vs baseline: 1.0257x; 1.0257x over previous
"""Trainium2 Bass kernel for nn_DCTLayer: 8x8 block DCT-II followed by its exact
inverse (torch_dct norm=None convention). The DCT->IDCT round trip is the
identity map in exact arithmetic, so the layer reduces to the block-layout
permutation (B, C, H, W) -> (B, C, 1, H, W) where out[b, c, 0] is the row-major
flatten of the (H/8, W/8, 8, 8) block view of the input. Computing the
permutation exactly is strictly more accurate than the reference's own fp32 FFT
round trip (rel err ~1e-7 against it).

Distribution (pure data parallelism over batch, 8 cores, no communication):
  - core k handles batches 4k..4k+4 = 12 images of 512x512 f32 (12 MiB).
  - Input viewed as [768, 4096]: each row chunk = 8 consecutive image rows
    (16 KiB, DRAM-contiguous) -> one SBUF partition.
  - On-chip shuffle per partition (vector engine, 4D access patterns):
    free-dim permutation (r, bw, c) -> (bw, r, c) with r=8, bw=64, c=8.
  - Output [768, 4096] is DRAM-contiguous per partition too, so both DMAs run
    at full descriptor efficiency. Binding resource: per-engine SDMA port rate
    (~26.5 GB/s x 16 engines) -> ~60 us conveyor per core.

Raw bass (no TileContext), minimal semaphore protocol:
  - all 6 tile loads issued up front, alternating the two HWDGE rings
    (SP=sync, ACT=scalar) so the DMA engines fill from both descriptor
    generators at once; the whole 96 KiB/partition input stays resident.
  - 2 half-tile shuffles per tile on vector, each into a dedicated out
    buffer column (96 KiB/partition out, no buffer reuse -> no waits).
  - stores split per half-tile across both rings behind the copies.
  - exactly one completion wait (gpsimd, single store sem) gates the NEFF
    epilogue; the wrapper re-clears all kernel sems at each execution's
    preamble, so no kernel-side sem cleanup is emitted at all.
"""

import numpy as np

_B, _C, _H, _W = 32, 3, 512, 512
_N_CORES = 8
_ROWS = (_B // _N_CORES) * _C * (_H // 8)  # 768 row chunks per core
_COLS = 8 * _W                             # 4096 f32 per chunk
_N_TILES = _ROWS // 128                    # 6 tiles of [128, 4096]

_nc_cache = None


def _build():
    import concourse.mybir as mybir
    from concourse import bacc

    nc = bacc.Bacc(
        "TRN2", target_bir_lowering=False, debug=False, num_devices=_N_CORES
    )
    x = nc.dram_tensor(
        "x", (_ROWS, _COLS), mybir.dt.float32, kind="ExternalInput"
    ).ap()
    y = nc.dram_tensor(
        "y", (_ROWS, _COLS), mybir.dt.float32, kind="ExternalOutput"
    ).ap()

    f32 = mybir.dt.float32
    with (
        nc.sbuf_tensor([128, _N_TILES * _COLS], f32) as tin,
        nc.sbuf_tensor([128, _N_TILES * _COLS], f32) as tout,
        nc.semaphore() as sem_ld_sp,   # loads on sync (SP ring)
        nc.semaphore() as sem_ld_act,  # loads on scalar (ACT ring)
        nc.semaphore() as sem_cp,      # vector copies
        nc.semaphore() as sem_st,      # stores, both rings
    ):
        # All 6 loads issued up front: L0,L2,L4 -> sync, L1,L3,L5 -> scalar.
        # tin column block t holds DRAM rows [128t, 128t+128).
        for t in range(_N_TILES):
            eng = nc.sync if t % 2 == 0 else nc.scalar
            sem = sem_ld_sp if t % 2 == 0 else sem_ld_act
            eng.dma_start(
                out=tin[:, t * _COLS:(t + 1) * _COLS],
                in_=x[t * 128:(t + 1) * 128, :],
                single_packet=True,
            ).then_inc(sem, 16)

        # Vector: per tile, 2 half-shuffles (bw split) into a dedicated out
        # buffer column (no reuse).
        for t in range(_N_TILES):
            sem = sem_ld_sp if t % 2 == 0 else sem_ld_act
            nc.vector.wait_ge(sem, 16 * (t // 2 + 1))
            src = tin[:, t * _COLS:(t + 1) * _COLS].rearrange(
                "p (r bw c) -> p bw r c", r=8, bw=64, c=8
            )
            dst = tout[:, t * _COLS:(t + 1) * _COLS].rearrange(
                "p (bw r c) -> p bw r c", bw=64, r=8, c=8
            )
            for s in range(2):
                bws = slice(s * 32, (s + 1) * 32)
                nc.vector.tensor_copy(out=dst[:, bws], in_=src[:, bws]).then_inc(
                    sem_cp, 1
                )

        # Stores: one full-tile store per tile, alternating rings (fewer DMA
        # descriptors -> fewer profiling notifications -> less port
        # contention). Store t needs copy count 2t + 2.
        for t in range(_N_TILES):
            eng = nc.scalar if t % 2 == 0 else nc.sync
            eng.wait_ge(sem_cp, 2 * t + 2)
            eng.dma_start(
                out=y[t * 128:(t + 1) * 128, :],
                in_=tout[:, t * _COLS:(t + 1) * _COLS],
                single_packet=True,
            ).then_inc(sem_st, 16)

        # Single completion gate: all 12 stores (2 rings) counted on one sem.
        # The NEFF preamble re-clears the whole kernel sem range on every
        # execution, so no sem cleanup is needed here.
        nc.gpsimd.wait_ge(sem_st, 16 * _N_TILES)

        nc.compile()
    return nc


def kernel(x: np.ndarray) -> np.ndarray:
    from concourse import bass_utils

    global _nc_cache
    if _nc_cache is None:
        _nc_cache = _build()
    nc = _nc_cache

    x = np.ascontiguousarray(x, dtype=np.float32)
    assert x.shape == (_B, _C, _H, _W), x.shape
    xs = x.reshape(_N_CORES, _ROWS, _COLS)
    in_maps = [{"x": xs[k]} for k in range(_N_CORES)]
    res = bass_utils.run_bass_kernel_spmd(
        nc, in_maps, core_ids=list(range(_N_CORES))
    )
    ys = np.stack([res.results[k]["y"] for k in range(_N_CORES)], axis=0)
    return ys.reshape(_B, _C, 1, _H, _W)
